# revision 8
# baseline (speedup 1.0000x reference)
"""Trainium2 Bass kernel for 2-layer BiLSTM + classifier (nn_BiLSTM_45234595561814).

Strategy (8 NeuronCores, single SPMD launch, no collectives):
  - Each core q owns a 64-token window W_q = [64q, 64q+64) of T=512, FULL batch
    (B=64), and runs BOTH directions as 2 independent interleaved chains
    (hides the per-step cross-engine dependency chain).
  - Sequence parallelism via truncated warmup: LSTM state decays ~0.5/step, so
    a chain zero-initialized WARM steps before its window converges to the
    exact state (err ~ WARM * 2^-WARM).  Layer-0 chains span
    [window-W, window+64+W) so layer-1 warmups are fed locally -> the
    (L0,L1) cascade self-warms; no cross-core exchange anywhere.
  - One-tanh trick: i,f,o weight rows pre-scaled by 0.5 so sigmoid(z) =
    0.5*(1+tanh(z/2)) needs only tanh -> ONE ACT op for all 4 gates.
    State kept doubled (C=2c, hh=2h); cell update is 3 scalar_tensor_tensor
    DVE ops + 1 for hh.  Whh pre-scaled by extra 0.5 to absorb hh=2h.
  - L0 input projection fused into the per-step PSUM accumulation; xaug and
    wihT0 zero-padded to K=128 so LDWEIGHTS takes the FWL fast path.
  - L1 projection precomputed into DRAM (bf16) and streamed back; accumulated
    into gate PSUM via bf16 identity-matmul (1 cycle/row vs 4 for f32).
  - Gate tanh is a single ACT instr over all 4 gates; gate/cell tiles are
    bf16 (faster ACT writes + DVE 2x mode); per-step gate PSUM tiles are
    padded to a full PSUM bank so ACT reads never share a bank with the
    next step's matmul writes.
  - Pad tokens (outside [0,512)) handled exactly: x/ones rows zero keep state
    at 0 through leading pads; an L1 control row drives the i-gate preact to
    -30000 on pad tokens so pad xg1 cannot perturb state.
  - Classifier is fully local; final GEMM emitted transposed (tokens on
    partitions), tanh batched 4 token-blocks per ACT, one strided DMA per 4.

kernel(**inputs) takes the FULL inputs and returns the FULL [64,512,64] f32
output.  Self-contained: hardcodes all shapes; no sibling imports.
"""

import os

import numpy as np
import ml_dtypes

import concourse.bass as bass
import concourse.mybir as mybir
import concourse.tile as tile
from concourse import bacc
from concourse.bass_utils import run_bass_kernel_spmd

bf16 = ml_dtypes.bfloat16
F32, BF16 = mybir.dt.float32, mybir.dt.bfloat16
AluOp = mybir.AluOpType
ACT_TANH = mybir.ActivationFunctionType.Tanh
ACT_RELU = mybir.ActivationFunctionType.Relu

H = 128          # rnn size
B = 64           # batch
T = 512          # seq len
D = 64           # input size
NC = 8           # cores
WIN = T // NC    # tokens per core window = 64
WARM = int(os.environ.get("BILSTM_WARM", "8"))
STATE_BF16 = os.environ.get("BILSTM_STATE_BF16", "0") == "1"
SPAN0 = WIN + 2 * WARM   # L0 chain steps (slots)
SPAN1 = WIN + WARM       # L1 chain steps
PADKILL = -30000.0
KP = 128         # padded contraction dim for L0 inproj (D+1 -> 128, FWL)

_CACHE = {}


def _build_program():
    nc = bacc.Bacc(None, target_bir_lowering=False)

    # ---------------- I/O declarations ----------------
    ei = lambda name, shape, dt=BF16: nc.dram_tensor(name, shape, dt, kind="ExternalInput")
    xaug = ei("xaug", [KP, SPAN0 * B])             # rows 0..63 x.T, row 64 ones, rest 0
    ctl1 = ei("ctl1", [2, SPAN0 * B])              # row0 valid, row1 padkill indicator
    wihT0 = {d: ei(f"wihT0{d}", [KP, 4 * H]) for d in "fb"}
    whhT0 = {d: ei(f"whhT0{d}", [H, 4 * H]) for d in "fb"}
    whhT1 = {d: ei(f"whhT1{d}", [H, 4 * H]) for d in "fb"}
    wih1Ta = {d: ei(f"wih1Ta{d}", [H, 4 * H]) for d in "fb"}   # y0f K-tile
    wih1Tb = {d: ei(f"wih1Tb{d}", [H, 4 * H]) for d in "fb"}   # y0b K-tile
    ctlT1 = {d: ei(f"ctlT1{d}", [2, 4 * H]) for d in "fb"}     # bias row + padkill row
    idn = ei("idn", [H, H])
    w1Ta = ei("w1Ta", [H, 2 * H])   # (0.5*W1).T rows 0:128  -> [128, 256]
    w1Tb = ei("w1Tb", [H, 2 * H])   # rows 128:256
    b1row = ei("b1row", [1, 2 * H])
    w2Ta = ei("w2Ta", [H, D])       # W2.T rows 0:128 -> [128, 64]
    w2Tb = ei("w2Tb", [H, D])
    b2row = ei("b2row", [1, D])
    out = nc.dram_tensor("out", [WIN * B, D], F32, kind="ExternalOutput")

    with tile.TileContext(nc) as tc:
        with tc.tile_pool(name="singles", bufs=1) as singles, \
             tc.tile_pool(name="state", bufs=1) as state, \
             tc.tile_pool(name="tpool", bufs=3) as tpool, \
             tc.tile_pool(name="vpool", bufs=3) as vpool, \
             tc.tile_pool(name="clssb", bufs=3) as clssb, \
             tc.tile_pool(name="psA", bufs=2, space="PSUM") as psA, \
             tc.tile_pool(name="psB", bufs=2, space="PSUM") as psB, \
             tc.tile_pool(name="psP", bufs=2, space="PSUM") as psP:

            # ---------------- load constants ----------------
            def load(src, shape, dt=BF16):
                t = singles.tile(shape, dt, name=src.name, tag=src.name)
                nc.sync.dma_start(out=t[:], in_=src[:])
                return t

            # xaug split into 4 column chunks so the first L0 pairs can start
            # before the whole input lands
            xaug_t = singles.tile([KP, SPAN0 * B], BF16, name="xaug", tag="xaug")
            XCH = SPAN0 * B // 4
            for i in range(4):
                nc.sync.dma_start(out=xaug_t[:, i * XCH:(i + 1) * XCH],
                                  in_=xaug[:, i * XCH:(i + 1) * XCH])
            ctl1_t = load(ctl1, [2, SPAN0 * B])
            wihT0_t = {d: load(wihT0[d], [KP, 4 * H]) for d in "fb"}
            whhT0_t = {d: load(whhT0[d], [H, 4 * H]) for d in "fb"}
            whhT1_t = {d: load(whhT1[d], [H, 4 * H]) for d in "fb"}
            wih1Ta_t = {d: load(wih1Ta[d], [H, 4 * H]) for d in "fb"}
            wih1Tb_t = {d: load(wih1Tb[d], [H, 4 * H]) for d in "fb"}
            ctlT1_t = {d: load(ctlT1[d], [2, 4 * H]) for d in "fb"}
            idn_t = load(idn, [H, H])
            w1Ta_t = load(w1Ta, [H, 2 * H])
            w1Tb_t = load(w1Tb, [H, 2 * H])
            b1row_t = load(b1row, [1, 2 * H])
            w2Ta_t = load(w2Ta, [H, D])
            w2Tb_t = load(w2Tb, [H, D])
            b2row_t = load(b2row, [1, D])

            # ---------------- persistent state ----------------
            y0 = {d: state.tile([H, SPAN0 * B], BF16, name=f"y0{d}", tag=f"y0{d}") for d in "fb"}
            y1 = {d: state.tile([H, SPAN1 * B], BF16, name=f"y1{d}", tag=f"y1{d}") for d in "fb"}
            h00 = state.tile([H, B], BF16, name="h00", tag="h00")
            nc.vector.memset(h00[:], 0.0)

            # L1 input projection staged fully in SBUF (pair-packed layout:
            # pair q at cols [q*8B,(q+1)*8B), col = g*2B + (slot%2)*B + b)
            xg1 = {d: state.tile([H, SPAN1 * 4 * B], BF16, name=f"xg1{d}", tag=f"xg1{d}") for d in "fb"}

            # ---------------- generic LSTM step ----------------
            # PAIR-PACKED psum: one [H, 8B] tile (exactly 1 PSUM bank) holds
            # TWO consecutive chain steps, gate-major-paired layout:
            #   col = g*2B + half*B + b  (half = which step of the pair)
            # This halves the inproj matmul/LDWEIGHTS count (N=128 per gate
            # covers both steps).  The ACT(step s) vs hh-matmul(step s+1)
            # same-bank hazard is subsumed by the recurrence data dependency
            # (hh(s+1) needs y(s) which needs ACT(s)), so no extra stalls.
            # T-tile col order: [o | i | f | g]*B + C in cols 4B:5B (written by
            # the PREVIOUS step's c-update into THIS step's tile).
            SDT = BF16 if STATE_BF16 else F32

            # both chains' cell updates on DVE (the "Pool" EngineType is the
            # GpSimd DSP on trn2: no STT opcode, no PSUM access)
            VENG = {"f": nc.vector, "b": nc.vector}

            def lstm_step2(lt, whh, hp, yout, cur, nxt_T):
                ctx = tc.high_priority(offset=150)
                ctx.__enter__()
                for d in "fb":
                    g_pair, half, _ = cur[d]
                    for g in range(4):
                        c0 = g * 2 * B + half * B
                        nc.tensor.matmul(g_pair[:, c0:c0 + B],
                                         whh[d][:, g * H:(g + 1) * H], hp[d],
                                         start=False, stop=True,
                                         skip_group_check=True)
                for d in "fb":
                    g_pair, half, Tt = cur[d]
                    gv = g_pair[:].rearrange("h (g tb) -> h g tb", g=4)
                    nc.scalar.activation(
                        Tt[:, 0:4 * B].rearrange("h (g b) -> h g b", g=4),
                        gv[:, :, half * B:(half + 1) * B], ACT_TANH)
                scr = {}
                for d in "fb":
                    Tt = cur[d][2]
                    scr[d] = vpool.tile([H, 2 * B], SDT, name="s" + lt + d, tag="s" + lt + d)
                    # scr = [(1+ti)*tg | (1+tf)*C] = [Bv | A]
                    VENG[d].scalar_tensor_tensor(scr[d][:], Tt[:, B:3 * B], 1.0,
                                                 Tt[:, 3 * B:5 * B], AluOp.add, AluOp.mult)
                for d in "fb":
                    VENG[d].scalar_tensor_tensor(nxt_T[d][:, 4 * B:5 * B], scr[d][:, B:2 * B],
                                                 0.5, scr[d][:, 0:B], AluOp.mult, AluOp.add)
                tc_t = {}
                for d in "fb":
                    tc_t[d] = vpool.tile([H, B], SDT, name="c" + lt + d, tag="c" + lt + d)
                    nc.scalar.activation(tc_t[d][:], nxt_T[d][:, 4 * B:5 * B], ACT_TANH, scale=0.5)
                for d in "fb":
                    Tt = cur[d][2]
                    VENG[d].scalar_tensor_tensor(yout[d], Tt[:, 0:B], 1.0, tc_t[d][:],
                                                 AluOp.add, AluOp.mult)
                ctx.__exit__(None, None, None)

            # ---------------- layer 0 (fused input projection) ----------------
            # chain step s uses pair p=s//2; psum half: f -> s%2, b -> 1-s%2
            # (chain b's pair covers slots descending but rhs is read in
            # ascending memory order).
            NP0 = SPAN0 // 2
            pend0 = {}       # (d, step) -> (g_pair, half, T tile)
            pT0 = {}         # (d, step) -> T tile

            def l0_pair(p, first=False):
                for d in "fb":
                    ps_pool = psA if d == "f" else psB
                    g_pair = ps_pool.tile([H, 8 * B], F32, name="g0" + d, tag="g" + d,
                                          bufs=3)
                    c0 = (2 * p) * B if d == "f" else (SPAN0 - 2 - 2 * p) * B
                    for g in range(4):
                        nc.tensor.matmul(g_pair[:, g * 2 * B:(g + 1) * 2 * B],
                                         wihT0_t[d][:, g * H:(g + 1) * H],
                                         xaug_t[:, c0:c0 + 2 * B],
                                         start=(g == 0), stop=False,
                                         skip_group_check=True)
                    for t in (0, 1):
                        step = 2 * p + t
                        half = t if d == "f" else 1 - t
                        t_t = tpool.tile([H, 5 * B], SDT, name="t0" + d, tag="t0" + d, bufs=4)
                        if first and step == 0:
                            nc.vector.memset(t_t[:, 4 * B:5 * B], 0.0)
                        pend0[(d, step)] = (g_pair, half, t_t)
                        pT0[(d, step)] = t_t

            # ---------------- layer-1 projection (SBUF staged) ----------------
            # xg1[d] covers local slots [0, SPAN1) of [lo, lo+SPAN1); layout is
            # gate-major-PAIRED: pair q (local slots 2q,2q+1) occupies cols
            # [q*8B,(q+1)*8B) with col = g*2B + (slot%2)*B + b, matching the
            # psum pair-tile layout so one N=512 identity matmul injects a
            # whole pair.
            CH = 512                      # psum cols per chunk = 8 slots
            SLOTS_PER_CH = CH // B
            NCH = (SPAN1 + SLOTS_PER_CH - 1) // SLOTS_PER_CH
            # chain f consumes slots [0, SPAN1); chain b consumes [WARM, SPAN0)
            proj_lo = {"f": 0, "b": WARM}

            def l1_proj_chunk(dirn, j):
                lo = proj_lo[dirn]
                s0 = j * SLOTS_PER_CH
                nsl = min(SLOTS_PER_CH, SPAN1 - s0)
                ncols = nsl * B
                npr = nsl // 2
                col0 = (lo + s0) * B                        # into y0/ctl tiles
                base = s0 * 4 * B
                st3 = xg1[dirn][:, base:base + nsl * 4 * B].rearrange(
                    "h (pr c) -> h pr c", pr=npr)
                for g in range(4):
                    p = psP.tile([H, CH], F32, name="pp", tag="pp")
                    nc.tensor.matmul(p[:, 0:ncols], wih1Ta_t[dirn][:, g * H:(g + 1) * H],
                                     y0["f"][:, col0:col0 + ncols], start=True, stop=False)
                    nc.tensor.matmul(p[:, 0:ncols], wih1Tb_t[dirn][:, g * H:(g + 1) * H],
                                     y0["b"][:, col0:col0 + ncols], start=False, stop=False)
                    nc.tensor.matmul(p[:, 0:ncols], ctlT1_t[dirn][:, g * H:(g + 1) * H],
                                     ctl1_t[:, col0:col0 + ncols], start=False, stop=True)
                    # scatter gate g into gate-major-paired layout (DVE: the
                    # Pool engine cannot read PSUM, ACT is the critical engine)
                    nc.vector.tensor_copy(st3[:, :, g * 2 * B:(g + 1) * 2 * B],
                                          p[:, 0:ncols].rearrange("h (pr tb) -> h pr tb", pr=npr))

            # proj chunk (dirn, j) needs y0 f+b for abs slots
            # [lo+8j, lo+8j+8): f-chain step lo+8j+7, b-chain step
            # SPAN0-1-(lo+8j)  ->  ready after L0 step max(...)
            proj_ready = {}
            for dirn in "fb":
                lo = proj_lo[dirn]
                for j in range(NCH):
                    s0, s1 = lo + 8 * j, min(lo + 8 * j + SLOTS_PER_CH, lo + SPAN1) - 1
                    r = max(s1, SPAN0 - 1 - s0)
                    proj_ready.setdefault(r, []).append((dirn, j))

            l0_pair(0, first=True)
            for step in range(SPAN0):
                if step % 2 == 0 and step // 2 + 1 < NP0:
                    l0_pair(step // 2 + 1)
                if step == SPAN0 - 1:
                    for d in "fb":
                        pT0[(d, SPAN0)] = tpool.tile([H, 5 * B], SDT, name="t0" + d,
                                                     tag="t0" + d, bufs=4)
                pf, pb = step, SPAN0 - 1 - step
                hp = {"f": h00[:] if pf == 0 else y0["f"][:, (pf - 1) * B:pf * B],
                      "b": h00[:] if pf == 0 else y0["b"][:, (pb + 1) * B:(pb + 2) * B]}
                lstm_step2("0", whhT0_t, hp,
                           {"f": y0["f"][:, pf * B:(pf + 1) * B],
                            "b": y0["b"][:, pb * B:(pb + 1) * B]},
                           {"f": pend0.pop(("f", step)), "b": pend0.pop(("b", step))},
                           {"f": pT0[("f", step + 1)], "b": pT0[("b", step + 1)]})
                # emit L1-projection chunks as soon as their y0 slots complete
                for dirn, j in proj_ready.get(step, ()):
                    l1_proj_chunk(dirn, j)

            # ---------------- layer 1 recurrence ----------------
            NP1 = SPAN1 // 2
            pend1 = {}
            pT1 = {}

            def l1_pair(p, first=False):
                for d in "fb":
                    ps_pool = psA if d == "f" else psB
                    g_pair = ps_pool.tile([H, 8 * B], F32, name="g1" + d, tag="g" + d,
                                          bufs=3)
                    q = p if d == "f" else NP1 - 1 - p      # xg1 pair index
                    nc.tensor.matmul(g_pair[:], idn_t[:],
                                     xg1[d][:, q * 8 * B:(q + 1) * 8 * B],
                                     start=True, stop=False, skip_group_check=True)
                    for t in (0, 1):
                        step = 2 * p + t
                        half = t if d == "f" else 1 - t
                        t_t = tpool.tile([H, 5 * B], SDT, name="t1" + d, tag="t1" + d, bufs=4)
                        if first and step == 0:
                            nc.vector.memset(t_t[:, 4 * B:5 * B], 0.0)
                        pend1[(d, step)] = (g_pair, half, t_t)
                        pT1[(d, step)] = t_t

            # ---------------- classifier (interleaved into L1) ----------------
            # window tokens: slot s in [WARM, WARM+WIN)
            #   y1f idx = s        -> cols [WARM*B, (WARM+WIN)*B)
            #   y1b idx = s - WARM -> cols [0, WIN*B)
            # ones: ctl1 row0 cols [WARM*B ...)
            NTOK = WIN * B                      # 4096 columns
            h1 = [clssb.tile([H, NTOK], BF16, name="h1a", tag="h1a", bufs=1),
                  clssb.tile([H, NTOK], BF16, name="h1b", tag="h1b", bufs=1)]

            def cls_chunk(c0):
                for m in range(2):
                    p = psP.tile([H, CH], F32, name="pc", tag="pp")
                    nc.tensor.matmul(p[:], w1Ta_t[:, m * H:(m + 1) * H],
                                     y1["f"][:, WARM * B + c0:WARM * B + c0 + CH],
                                     start=True, stop=False)
                    nc.tensor.matmul(p[:], w1Tb_t[:, m * H:(m + 1) * H],
                                     y1["b"][:, c0:c0 + CH], start=False, stop=False)
                    nc.tensor.matmul(p[:], b1row_t[:, m * H:(m + 1) * H],
                                     ctl1_t[0:1, WARM * B + c0:WARM * B + c0 + CH],
                                     start=False, stop=True)
                    nc.scalar.activation(h1[m][:, c0:c0 + CH], p[:], ACT_RELU)
                # final GEMM transposed: out[tok, d] (tokens on partitions);
                # 4 token-blocks batched per psum tile -> 1 tanh ACT + DMA per 4
                p = psP.tile([H, 4 * D], F32, name="po", tag="pp")
                for j in range(4):
                    cj = c0 + j * H
                    nc.tensor.matmul(p[:, j * D:(j + 1) * D], h1[0][:, cj:cj + H],
                                     w2Ta_t[:], start=True, stop=False)
                    nc.tensor.matmul(p[:, j * D:(j + 1) * D], h1[1][:, cj:cj + H],
                                     w2Tb_t[:], start=False, stop=False)
                    nc.tensor.matmul(p[:, j * D:(j + 1) * D],
                                     ctl1_t[0:1, WARM * B + cj:WARM * B + cj + H],
                                     b2row_t[:], start=False, stop=True)
                o_t = clssb.tile([H, 4 * D], F32, name="ot", tag="ot")
                nc.scalar.activation(o_t[:], p[:], ACT_TANH)
                for j in range(4):
                    cj = c0 + j * H
                    nc.sync.dma_start(out=out[cj:cj + H, :],
                                      in_=o_t[:, j * D:(j + 1) * D])

            # chunk c0 covers window slots [c0/B, c0/B+8): needs y1f step
            # WARM+c0/B+7 and y1b step SPAN1-1-c0/B
            cls_ready = {}
            for c0 in range(0, NTOK, CH):
                s0 = c0 // B
                r = max(WARM + s0 + SLOTS_PER_CH - 1, SPAN1 - 1 - s0)
                cls_ready.setdefault(r, []).append(c0)

            l1_pair(0, first=True)
            for step in range(SPAN1):
                if step % 2 == 0 and step // 2 + 1 < NP1:
                    l1_pair(step // 2 + 1)
                if step == SPAN1 - 1:
                    for d in "fb":
                        pT1[(d, SPAN1)] = tpool.tile([H, 5 * B], SDT, name="t1" + d,
                                                     tag="t1" + d, bufs=4)
                pf = step
                pb = SPAN1 - 1 - step
                hp = {"f": h00[:] if pf == 0 else y1["f"][:, (pf - 1) * B:pf * B],
                      "b": h00[:] if pf == 0 else y1["b"][:, (pb + 1) * B:(pb + 2) * B]}
                lstm_step2("1", whhT1_t, hp,
                           {"f": y1["f"][:, pf * B:(pf + 1) * B],
                            "b": y1["b"][:, pb * B:(pb + 1) * B]},
                           {"f": pend1.pop(("f", step)), "b": pend1.pop(("b", step))},
                           {"f": pT1[("f", step + 1)], "b": pT1[("b", step + 1)]})
                for c0 in cls_ready.get(step, ()):
                    cls_chunk(c0)

    nc.compile()
    return nc


# ======================= host side =======================

def _prep_weights(inp):
    """Returns dict of np arrays shared by all cores (bf16).

    Gate row-blocks reordered from reference [i,f,g,o] to device [o,i,f,g];
    i,f,o rows scaled 0.5 (one-tanh trick)."""
    H_ = H
    sr = np.full((4 * H_, 1), 0.5, np.float32)
    sr[2 * H_:3 * H_] = 1.0

    def reorder(a):           # rows [i,f,g,o] -> [o,i,f,g]
        return np.concatenate([a[3 * H_:], a[:H_], a[H_:2 * H_], a[2 * H_:3 * H_]], 0)

    w = {}
    for d, tag in (("f", "0"), ("b", "1")):
        Wih, Whh = inp[f"Wih0{tag}"], inp[f"Whh0{tag}"]
        bias = inp[f"bih0{tag}"] + inp[f"bhh0{tag}"]
        wihT = reorder(np.concatenate([Wih * sr, (bias[:, None] * sr)], 1)).T  # [65, 4H]
        w[f"wihT0{d}"] = np.concatenate(
            [wihT, np.zeros((KP - D - 1, 4 * H_), np.float32)], 0).astype(bf16)
        w[f"whhT0{d}"] = reorder(Whh * sr * 0.5).T.astype(bf16)
        Wih1, Whh1 = inp[f"Wih1{tag}"], inp[f"Whh1{tag}"]
        bias1 = reorder((inp[f"bih1{tag}"] + inp[f"bhh1{tag}"])[:, None] * sr).T
        w[f"whhT1{d}"] = reorder(Whh1 * sr * 0.5).T.astype(bf16)
        w[f"wih1Ta{d}"] = reorder(Wih1[:, :H] * sr * 0.5).T.astype(bf16)
        w[f"wih1Tb{d}"] = reorder(Wih1[:, H:] * sr * 0.5).T.astype(bf16)
        padkill = np.zeros((1, 4 * H), np.float32)
        padkill[0, H:2 * H] = PADKILL      # i-gate block (device order [o,i,f,g])
        w[f"ctlT1{d}"] = np.concatenate([bias1, padkill], 0).astype(bf16)
    w["idn"] = np.eye(H, dtype=np.float32).astype(bf16)
    w["w1Ta"] = (0.5 * inp["W1"][:, :H]).T.astype(bf16)
    w["w1Tb"] = (0.5 * inp["W1"][:, H:]).T.astype(bf16)
    w["b1row"] = inp["b1"][None, :].astype(bf16)
    w["w2Ta"] = inp["W2"][:, :H].T.astype(bf16)
    w["w2Tb"] = inp["W2"][:, H:].T.astype(bf16)
    w["b2row"] = inp["b2"][None, :].astype(bf16)
    return w


def _per_core_inputs(x, q):
    """x: [B, T, D] f32.  Builds xaug [KP, SPAN0*B] and ctl1 [2, SPAN0*B]."""
    t0 = WIN * q - WARM
    xaug = np.zeros((KP, SPAN0 * B), np.float32)
    ctl = np.zeros((2, SPAN0 * B), np.float32)
    for s in range(SPAN0):
        t = t0 + s
        sl = slice(s * B, (s + 1) * B)
        if 0 <= t < T:
            xaug[:D, sl] = x[:, t, :].T
            xaug[D, sl] = 1.0
            ctl[0, sl] = 1.0
        else:
            ctl[1, sl] = 1.0
    return xaug.astype(bf16), ctl.astype(bf16)


def _get_program():
    if "nc" not in _CACHE:
        _CACHE["nc"] = _build_program()
    return _CACHE["nc"]


def _run(inputs, trace=False):
    inp = {k: np.asarray(v) for k, v in inputs.items()}
    nc = _get_program()
    w = _prep_weights(inp)
    x = inp["x"].astype(np.float32)
    in_maps = []
    for q in range(NC):
        xaug, ctl = _per_core_inputs(x, q)
        m = dict(w)
        m["xaug"] = xaug
        m["ctl1"] = ctl
        in_maps.append(m)
    res = run_bass_kernel_spmd(nc, in_maps, list(range(NC)), trace=trace)
    outp = np.zeros((B, T, D), np.float32)
    for q in range(NC):
        o = res.results[q]["out"].reshape(WIN, B, D)        # [tok, b, d]
        outp[:, WIN * q:WIN * (q + 1), :] = o.transpose(1, 0, 2)
    return outp, res


def kernel(**inputs):
    out, _ = _run(inputs, trace=False)
    return out



# revision 12
# speedup vs baseline: 1.0214x; 1.0214x over previous
"""Trainium2 Bass kernel for 2-layer BiLSTM + classifier (nn_BiLSTM_45234595561814).

Strategy (8 NeuronCores, single SPMD launch, no collectives):
  - Each core q owns a 64-token window W_q = [64q, 64q+64) of T=512, FULL batch
    (B=64), and runs BOTH directions as 2 independent interleaved chains
    (hides the per-step cross-engine dependency chain).
  - Sequence parallelism via truncated warmup: LSTM state decays ~0.5/step, so
    a chain zero-initialized WARM steps before its window converges to the
    exact state (err ~ WARM * 2^-WARM).  Layer-0 chains span
    [window-W, window+64+W) so layer-1 warmups are fed locally -> the
    (L0,L1) cascade self-warms; no cross-core exchange anywhere.
  - One-tanh trick: i,f,o weight rows pre-scaled by 0.5 so sigmoid(z) =
    0.5*(1+tanh(z/2)) needs only tanh -> ONE ACT op for all 4 gates.
    State kept doubled (C=2c, hh=2h); cell update is 3 scalar_tensor_tensor
    DVE ops + 1 for hh.  Whh pre-scaled by extra 0.5 to absorb hh=2h.
  - L0 input projection fused into the per-step PSUM accumulation; xaug and
    wihT0 zero-padded to K=128 so LDWEIGHTS takes the FWL fast path.
  - L1 projection precomputed into DRAM (bf16) and streamed back; accumulated
    into gate PSUM via bf16 identity-matmul (1 cycle/row vs 4 for f32).
  - Gate tanh is a single ACT instr over all 4 gates; gate/cell tiles are
    bf16 (faster ACT writes + DVE 2x mode); per-step gate PSUM tiles are
    padded to a full PSUM bank so ACT reads never share a bank with the
    next step's matmul writes.
  - Pad tokens (outside [0,512)) handled exactly: x/ones rows zero keep state
    at 0 through leading pads; an L1 control row drives the i-gate preact to
    -30000 on pad tokens so pad xg1 cannot perturb state.
  - Classifier is fully local; final GEMM emitted transposed (tokens on
    partitions), tanh batched 4 token-blocks per ACT, one strided DMA per 4.

kernel(**inputs) takes the FULL inputs and returns the FULL [64,512,64] f32
output.  Self-contained: hardcodes all shapes; no sibling imports.
"""

import os

import numpy as np
import ml_dtypes

import concourse.bass as bass
import concourse.mybir as mybir
import concourse.tile as tile
from concourse import bacc
from concourse.bass_utils import run_bass_kernel_spmd

bf16 = ml_dtypes.bfloat16
F32, BF16 = mybir.dt.float32, mybir.dt.bfloat16
AluOp = mybir.AluOpType
ACT_TANH = mybir.ActivationFunctionType.Tanh
ACT_RELU = mybir.ActivationFunctionType.Relu

H = 128          # rnn size
B = 64           # batch
T = 512          # seq len
D = 64           # input size
NC = 8           # cores
WIN = T // NC    # tokens per core window = 64
WARM = int(os.environ.get("BILSTM_WARM", "8"))
STATE_BF16 = os.environ.get("BILSTM_STATE_BF16", "0") == "1"
SPAN0 = WIN + 2 * WARM   # L0 chain steps (slots)
SPAN1 = WIN + WARM       # L1 chain steps
PADKILL = -30000.0
KP = 128         # padded contraction dim for L0 inproj (D+1 -> 128, FWL)

_CACHE = {}


def _build_program():
    nc = bacc.Bacc(None, target_bir_lowering=False)

    # ---------------- I/O declarations ----------------
    ei = lambda name, shape, dt=BF16: nc.dram_tensor(name, shape, dt, kind="ExternalInput")
    xaug = ei("xaug", [KP, SPAN0 * B])             # rows 0..63 x.T, row 64 ones, rest 0
    ctl1 = ei("ctl1", [2, SPAN0 * B])              # row0 valid, row1 padkill indicator
    wihT0 = {d: ei(f"wihT0{d}", [KP, 4 * H]) for d in "fb"}
    whhT0 = {d: ei(f"whhT0{d}", [H, 4 * H]) for d in "fb"}
    whhT1 = {d: ei(f"whhT1{d}", [H, 4 * H]) for d in "fb"}
    wih1Ta = {d: ei(f"wih1Ta{d}", [H, 4 * H]) for d in "fb"}   # y0f K-tile
    wih1Tb = {d: ei(f"wih1Tb{d}", [H, 4 * H]) for d in "fb"}   # y0b K-tile
    ctlT1 = {d: ei(f"ctlT1{d}", [2, 4 * H]) for d in "fb"}     # bias row + padkill row
    idn = ei("idn", [H, H])
    w1Ta = ei("w1Ta", [H, 2 * H])   # (0.5*W1).T rows 0:128  -> [128, 256]
    w1Tb = ei("w1Tb", [H, 2 * H])   # rows 128:256
    b1row = ei("b1row", [1, 2 * H])
    w2Ta = ei("w2Ta", [H, D])       # W2.T rows 0:128 -> [128, 64]
    w2Tb = ei("w2Tb", [H, D])
    b2row = ei("b2row", [1, D])
    out = nc.dram_tensor("out", [WIN * B, D], F32, kind="ExternalOutput")

    with tile.TileContext(nc) as tc:
        with tc.tile_pool(name="singles", bufs=1) as singles, \
             tc.tile_pool(name="state", bufs=1) as state, \
             tc.tile_pool(name="tpool", bufs=3) as tpool, \
             tc.tile_pool(name="vpool", bufs=3) as vpool, \
             tc.tile_pool(name="clssb", bufs=3) as clssb, \
             tc.tile_pool(name="psA", bufs=2, space="PSUM") as psA, \
             tc.tile_pool(name="psB", bufs=2, space="PSUM") as psB, \
             tc.tile_pool(name="psP", bufs=2, space="PSUM") as psP:

            # ---------------- load constants ----------------
            def load(src, shape, dt=BF16):
                t = singles.tile(shape, dt, name=src.name, tag=src.name)
                nc.sync.dma_start(out=t[:], in_=src[:])
                return t

            # xaug split into column chunks so the first L0 pairs can start
            # before the whole input lands; L0 weights issued right after
            # chunk 0, everything else after.
            xaug_t = singles.tile([KP, SPAN0 * B], BF16, name="xaug", tag="xaug")
            XCH = SPAN0 * B // 4
            # f chain reads cols ascending (chunk 0 first), b chain descending
            # (chunk 3 first)
            for i in (0, 3):
                nc.sync.dma_start(out=xaug_t[:, i * XCH:(i + 1) * XCH],
                                  in_=xaug[:, i * XCH:(i + 1) * XCH])
            wihT0_t = {d: load(wihT0[d], [KP, 4 * H]) for d in "fb"}
            whhT0_t = {d: load(whhT0[d], [H, 4 * H]) for d in "fb"}
            for i in (1, 2):
                nc.sync.dma_start(out=xaug_t[:, i * XCH:(i + 1) * XCH],
                                  in_=xaug[:, i * XCH:(i + 1) * XCH])
            ctl1_t = load(ctl1, [2, SPAN0 * B])
            whhT1_t = {d: load(whhT1[d], [H, 4 * H]) for d in "fb"}
            wih1Ta_t = {d: load(wih1Ta[d], [H, 4 * H]) for d in "fb"}
            wih1Tb_t = {d: load(wih1Tb[d], [H, 4 * H]) for d in "fb"}
            ctlT1_t = {d: load(ctlT1[d], [2, 4 * H]) for d in "fb"}
            idn_t = load(idn, [H, H])
            w1Ta_t = load(w1Ta, [H, 2 * H])
            w1Tb_t = load(w1Tb, [H, 2 * H])
            b1row_t = load(b1row, [1, 2 * H])
            w2Ta_t = load(w2Ta, [H, D])
            w2Tb_t = load(w2Tb, [H, D])
            b2row_t = load(b2row, [1, D])

            # ---------------- persistent state ----------------
            y0 = {d: state.tile([H, SPAN0 * B], BF16, name=f"y0{d}", tag=f"y0{d}") for d in "fb"}
            y1 = {d: state.tile([H, SPAN1 * B], BF16, name=f"y1{d}", tag=f"y1{d}") for d in "fb"}
            h00 = state.tile([H, B], BF16, name="h00", tag="h00")
            nc.vector.memset(h00[:], 0.0)

            # L1 input projection staged fully in SBUF (pair-packed layout:
            # pair q at cols [q*8B,(q+1)*8B), col = g*2B + (slot%2)*B + b)
            xg1 = {d: state.tile([H, SPAN1 * 4 * B], BF16, name=f"xg1{d}", tag=f"xg1{d}") for d in "fb"}

            # ---------------- generic LSTM step ----------------
            # PAIR-PACKED psum: one [H, 8B] tile (exactly 1 PSUM bank) holds
            # TWO consecutive chain steps, gate-major-paired layout:
            #   col = g*2B + half*B + b  (half = which step of the pair)
            # This halves the inproj matmul/LDWEIGHTS count (N=128 per gate
            # covers both steps).  The ACT(step s) vs hh-matmul(step s+1)
            # same-bank hazard is subsumed by the recurrence data dependency
            # (hh(s+1) needs y(s) which needs ACT(s)), so no extra stalls.
            # T-tile col order: [o | i | f | g]*B + C in cols 4B:5B (written by
            # the PREVIOUS step's c-update into THIS step's tile).
            SDT = BF16 if STATE_BF16 else F32

            # both chains' cell updates on DVE (the "Pool" EngineType is the
            # GpSimd DSP on trn2: no STT opcode, no PSUM access)
            VENG = {"f": nc.vector, "b": nc.vector}

            def lstm_step2(lt, whh, hp, yout, cur, nxt_T):
                ctx = tc.high_priority(offset=150)
                ctx.__enter__()
                for d in "fb":
                    g_pair, half, _ = cur[d]
                    for g in range(4):
                        c0 = g * 2 * B + half * B
                        nc.tensor.matmul(g_pair[:, c0:c0 + B],
                                         whh[d][:, g * H:(g + 1) * H], hp[d],
                                         start=False, stop=True,
                                         skip_group_check=True)
                for d in "fb":
                    g_pair, half, Tt = cur[d]
                    gv = g_pair[:].rearrange("h (g tb) -> h g tb", g=4)
                    nc.scalar.activation(
                        Tt[:, 0:4 * B].rearrange("h (g b) -> h g b", g=4),
                        gv[:, :, half * B:(half + 1) * B], ACT_TANH)
                scr = {}
                for d in "fb":
                    Tt = cur[d][2]
                    scr[d] = vpool.tile([H, 2 * B], SDT, name="s" + lt + d, tag="s" + lt + d)
                    # scr = [(1+ti)*tg | (1+tf)*C] = [Bv | A]
                    VENG[d].scalar_tensor_tensor(scr[d][:], Tt[:, B:3 * B], 1.0,
                                                 Tt[:, 3 * B:5 * B], AluOp.add, AluOp.mult)
                for d in "fb":
                    VENG[d].scalar_tensor_tensor(nxt_T[d][:, 4 * B:5 * B], scr[d][:, B:2 * B],
                                                 0.5, scr[d][:, 0:B], AluOp.mult, AluOp.add)
                tc_t = {}
                for d in "fb":
                    tc_t[d] = vpool.tile([H, B], SDT, name="c" + lt + d, tag="c" + lt + d)
                    nc.scalar.activation(tc_t[d][:], nxt_T[d][:, 4 * B:5 * B], ACT_TANH, scale=0.5)
                for d in "fb":
                    Tt = cur[d][2]
                    VENG[d].scalar_tensor_tensor(yout[d], Tt[:, 0:B], 1.0, tc_t[d][:],
                                                 AluOp.add, AluOp.mult)
                ctx.__exit__(None, None, None)

            # ---------------- layer 0 (fused input projection) ----------------
            # chain step s uses pair p=s//2; psum half: f -> s%2, b -> 1-s%2
            # (chain b's pair covers slots descending but rhs is read in
            # ascending memory order).
            NP0 = SPAN0 // 2
            pend0 = {}       # (d, step) -> (g_pair, half, T tile)
            pT0 = {}         # (d, step) -> T tile

            def l0_pair(p, first=False):
                for d in "fb":
                    ps_pool = psA if d == "f" else psB
                    g_pair = ps_pool.tile([H, 8 * B], F32, name="g0" + d, tag="g" + d,
                                          bufs=3)
                    c0 = (2 * p) * B if d == "f" else (SPAN0 - 2 - 2 * p) * B
                    for g in range(4):
                        nc.tensor.matmul(g_pair[:, g * 2 * B:(g + 1) * 2 * B],
                                         wihT0_t[d][:, g * H:(g + 1) * H],
                                         xaug_t[:, c0:c0 + 2 * B],
                                         start=(g == 0), stop=False,
                                         skip_group_check=True)
                    for t in (0, 1):
                        step = 2 * p + t
                        half = t if d == "f" else 1 - t
                        t_t = tpool.tile([H, 5 * B], SDT, name="t0" + d, tag="t0" + d, bufs=6)
                        if first and step == 0:
                            nc.vector.memset(t_t[:, 4 * B:5 * B], 0.0)
                        pend0[(d, step)] = (g_pair, half, t_t)
                        pT0[(d, step)] = t_t

            # ---------------- layer-1 projection (SBUF staged) ----------------
            # xg1[d] covers local slots [0, SPAN1) of [lo, lo+SPAN1); layout is
            # gate-major-PAIRED: pair q (local slots 2q,2q+1) occupies cols
            # [q*8B,(q+1)*8B) with col = g*2B + (slot%2)*B + b, matching the
            # psum pair-tile layout so one N=512 identity matmul injects a
            # whole pair.
            CH = 512                      # psum cols per chunk = 8 slots
            SLOTS_PER_CH = CH // B
            NCH = (SPAN1 + SLOTS_PER_CH - 1) // SLOTS_PER_CH
            # chain f consumes slots [0, SPAN1); chain b consumes [WARM, SPAN0)
            proj_lo = {"f": 0, "b": WARM}

            def l1_proj_chunk(dirn, j):
                lo = proj_lo[dirn]
                s0 = j * SLOTS_PER_CH
                nsl = min(SLOTS_PER_CH, SPAN1 - s0)
                ncols = nsl * B
                npr = nsl // 2
                col0 = (lo + s0) * B                        # into y0/ctl tiles
                base = s0 * 4 * B
                st3 = xg1[dirn][:, base:base + nsl * 4 * B].rearrange(
                    "h (pr c) -> h pr c", pr=npr)
                for g in range(4):
                    p = psP.tile([H, CH], F32, name="pp", tag="pp")
                    nc.tensor.matmul(p[:, 0:ncols], wih1Ta_t[dirn][:, g * H:(g + 1) * H],
                                     y0["f"][:, col0:col0 + ncols], start=True, stop=False)
                    nc.tensor.matmul(p[:, 0:ncols], wih1Tb_t[dirn][:, g * H:(g + 1) * H],
                                     y0["b"][:, col0:col0 + ncols], start=False, stop=False)
                    nc.tensor.matmul(p[:, 0:ncols], ctlT1_t[dirn][:, g * H:(g + 1) * H],
                                     ctl1_t[:, col0:col0 + ncols], start=False, stop=True)
                    # scatter gate g into gate-major-paired layout (DVE: the
                    # Pool engine cannot read PSUM, ACT is the critical engine)
                    nc.vector.tensor_copy(st3[:, :, g * 2 * B:(g + 1) * 2 * B],
                                          p[:, 0:ncols].rearrange("h (pr tb) -> h pr tb", pr=npr))

            # proj chunk (dirn, j) needs y0 f+b for abs slots
            # [lo+8j, lo+8j+8): f-chain step lo+8j+7, b-chain step
            # SPAN0-1-(lo+8j)  ->  ready after L0 step max(...).
            # Paced at <=1 chunk per 2 L0 steps so its 12 N=512 matmuls don't
            # head-of-line-block the recurrence's hh matmuls; the leftovers
            # drain deadline-ordered into the L1 loop.
            proj_info = []
            for dirn in "fb":
                lo = proj_lo[dirn]
                for j in range(NCH):
                    s0, s1 = lo + 8 * j, min(lo + 8 * j + SLOTS_PER_CH, lo + SPAN1) - 1
                    ready = max(s1, SPAN0 - 1 - s0)
                    if dirn == "f":           # consumed by l1_pair(4j) @ L1 step 8j-2
                        deadline = 8 * j - 2
                    else:                     # consumed when NP1-1-p in [4j,4j+4)
                        deadline = SPAN1 - 8 * j - 10
                    proj_info.append((ready, deadline, dirn, j))
            proj_l0 = {}          # L0 step -> chunk, paced 1 per 2 steps
            proj_rest = []
            slot = 48
            for ready, deadline, dirn, j in sorted(proj_info):
                if deadline < 0:
                    continue      # f0 / b8: emitted in the L1 prelude
                if slot < SPAN0 - 1 and ready <= slot:
                    proj_l0[max(slot, ready)] = (dirn, j)
                    slot += 2
                else:
                    proj_rest.append((deadline, dirn, j))
            proj_rest.sort()

            l0_pair(0, first=True)
            l0_pair(1)
            for step in range(SPAN0):
                # inproj 2 pairs ahead: runs in PE idle windows well before
                # the dependent steps, off the hh critical path
                if step % 2 == 0 and step // 2 + 2 < NP0:
                    l0_pair(step // 2 + 2)
                if step == SPAN0 - 1:
                    for d in "fb":
                        pT0[(d, SPAN0)] = tpool.tile([H, 5 * B], SDT, name="t0" + d,
                                                     tag="t0" + d, bufs=6)
                pf, pb = step, SPAN0 - 1 - step
                hp = {"f": h00[:] if pf == 0 else y0["f"][:, (pf - 1) * B:pf * B],
                      "b": h00[:] if pf == 0 else y0["b"][:, (pb + 1) * B:(pb + 2) * B]}
                lstm_step2("0", whhT0_t, hp,
                           {"f": y0["f"][:, pf * B:(pf + 1) * B],
                            "b": y0["b"][:, pb * B:(pb + 1) * B]},
                           {"f": pend0.pop(("f", step)), "b": pend0.pop(("b", step))},
                           {"f": pT0[("f", step + 1)], "b": pT0[("b", step + 1)]})
                if step in proj_l0:
                    l1_proj_chunk(*proj_l0[step])

            # ---------------- layer 1 recurrence ----------------
            NP1 = SPAN1 // 2
            pend1 = {}
            pT1 = {}

            def l1_pair(p, first=False):
                for d in "fb":
                    ps_pool = psA if d == "f" else psB
                    g_pair = ps_pool.tile([H, 8 * B], F32, name="g1" + d, tag="g" + d,
                                          bufs=3)
                    q = p if d == "f" else NP1 - 1 - p      # xg1 pair index
                    nc.tensor.matmul(g_pair[:], idn_t[:],
                                     xg1[d][:, q * 8 * B:(q + 1) * 8 * B],
                                     start=True, stop=False, skip_group_check=True)
                    for t in (0, 1):
                        step = 2 * p + t
                        half = t if d == "f" else 1 - t
                        t_t = tpool.tile([H, 5 * B], SDT, name="t1" + d, tag="t1" + d, bufs=6)
                        if first and step == 0:
                            nc.vector.memset(t_t[:, 4 * B:5 * B], 0.0)
                        pend1[(d, step)] = (g_pair, half, t_t)
                        pT1[(d, step)] = t_t

            # ---------------- classifier (interleaved into L1) ----------------
            # window tokens: slot s in [WARM, WARM+WIN)
            #   y1f idx = s        -> cols [WARM*B, (WARM+WIN)*B)
            #   y1b idx = s - WARM -> cols [0, WIN*B)
            # ones: ctl1 row0 cols [WARM*B ...)
            NTOK = WIN * B                      # 4096 columns
            h1 = [clssb.tile([H, NTOK], BF16, name="h1a", tag="h1a", bufs=1),
                  clssb.tile([H, NTOK], BF16, name="h1b", tag="h1b", bufs=1)]

            def cls_chunk(c0):
                for m in range(2):
                    p = psP.tile([H, CH], F32, name="pc", tag="pp")
                    nc.tensor.matmul(p[:], w1Ta_t[:, m * H:(m + 1) * H],
                                     y1["f"][:, WARM * B + c0:WARM * B + c0 + CH],
                                     start=True, stop=False)
                    nc.tensor.matmul(p[:], w1Tb_t[:, m * H:(m + 1) * H],
                                     y1["b"][:, c0:c0 + CH], start=False, stop=False)
                    nc.tensor.matmul(p[:], b1row_t[:, m * H:(m + 1) * H],
                                     ctl1_t[0:1, WARM * B + c0:WARM * B + c0 + CH],
                                     start=False, stop=True)
                    nc.scalar.activation(h1[m][:, c0:c0 + CH], p[:], ACT_RELU)
                # final GEMM transposed: out[tok, d] (tokens on partitions);
                # 4 token-blocks batched per psum tile -> 1 tanh ACT + DMA per 4
                p = psP.tile([H, 4 * D], F32, name="po", tag="pp")
                for j in range(4):
                    cj = c0 + j * H
                    nc.tensor.matmul(p[:, j * D:(j + 1) * D], h1[0][:, cj:cj + H],
                                     w2Ta_t[:], start=True, stop=False)
                    nc.tensor.matmul(p[:, j * D:(j + 1) * D], h1[1][:, cj:cj + H],
                                     w2Tb_t[:], start=False, stop=False)
                    nc.tensor.matmul(p[:, j * D:(j + 1) * D],
                                     ctl1_t[0:1, WARM * B + cj:WARM * B + cj + H],
                                     b2row_t[:], start=False, stop=True)
                o_t = clssb.tile([H, 4 * D], F32, name="ot", tag="ot")
                nc.scalar.activation(o_t[:], p[:], ACT_TANH)
                for j in range(4):
                    cj = c0 + j * H
                    nc.sync.dma_start(out=out[cj:cj + H, :],
                                      in_=o_t[:, j * D:(j + 1) * D])

            # chunk c0 covers window slots [c0/B, c0/B+8): needs y1f step
            # WARM+c0/B+7 and y1b step SPAN1-1-c0/B
            cls_ready = {}
            for c0 in range(0, NTOK, CH):
                s0 = c0 // B
                r = max(WARM + s0 + SLOTS_PER_CH - 1, SPAN1 - 1 - s0)
                cls_ready.setdefault(r, []).append(c0)

            # feed the first pairs of both L1 chains, then start L1
            l1_proj_chunk("f", 0)
            l1_proj_chunk("b", NCH - 1)
            l1_pair(0, first=True)
            l1_pair(1)
            nrest = 0
            for step in range(SPAN1):
                if step % 2 == 0 and nrest < len(proj_rest):
                    l1_proj_chunk(*proj_rest[nrest][1:])
                    nrest += 1
                if step % 2 == 0 and step // 2 + 2 < NP1:
                    l1_pair(step // 2 + 2)
                if step == SPAN1 - 1:
                    for d in "fb":
                        pT1[(d, SPAN1)] = tpool.tile([H, 5 * B], SDT, name="t1" + d,
                                                     tag="t1" + d, bufs=6)
                pf = step
                pb = SPAN1 - 1 - step
                hp = {"f": h00[:] if pf == 0 else y1["f"][:, (pf - 1) * B:pf * B],
                      "b": h00[:] if pf == 0 else y1["b"][:, (pb + 1) * B:(pb + 2) * B]}
                lstm_step2("1", whhT1_t, hp,
                           {"f": y1["f"][:, pf * B:(pf + 1) * B],
                            "b": y1["b"][:, pb * B:(pb + 1) * B]},
                           {"f": pend1.pop(("f", step)), "b": pend1.pop(("b", step))},
                           {"f": pT1[("f", step + 1)], "b": pT1[("b", step + 1)]})
                for c0 in cls_ready.get(step, ()):
                    cls_chunk(c0)

    nc.compile()
    return nc


# ======================= host side =======================

def _prep_weights(inp):
    """Returns dict of np arrays shared by all cores (bf16).

    Gate row-blocks reordered from reference [i,f,g,o] to device [o,i,f,g];
    i,f,o rows scaled 0.5 (one-tanh trick)."""
    H_ = H
    sr = np.full((4 * H_, 1), 0.5, np.float32)
    sr[2 * H_:3 * H_] = 1.0

    def reorder(a):           # rows [i,f,g,o] -> [o,i,f,g]
        return np.concatenate([a[3 * H_:], a[:H_], a[H_:2 * H_], a[2 * H_:3 * H_]], 0)

    w = {}
    for d, tag in (("f", "0"), ("b", "1")):
        Wih, Whh = inp[f"Wih0{tag}"], inp[f"Whh0{tag}"]
        bias = inp[f"bih0{tag}"] + inp[f"bhh0{tag}"]
        wihT = reorder(np.concatenate([Wih * sr, (bias[:, None] * sr)], 1)).T  # [65, 4H]
        w[f"wihT0{d}"] = np.concatenate(
            [wihT, np.zeros((KP - D - 1, 4 * H_), np.float32)], 0).astype(bf16)
        w[f"whhT0{d}"] = reorder(Whh * sr * 0.5).T.astype(bf16)
        Wih1, Whh1 = inp[f"Wih1{tag}"], inp[f"Whh1{tag}"]
        bias1 = reorder((inp[f"bih1{tag}"] + inp[f"bhh1{tag}"])[:, None] * sr).T
        w[f"whhT1{d}"] = reorder(Whh1 * sr * 0.5).T.astype(bf16)
        w[f"wih1Ta{d}"] = reorder(Wih1[:, :H] * sr * 0.5).T.astype(bf16)
        w[f"wih1Tb{d}"] = reorder(Wih1[:, H:] * sr * 0.5).T.astype(bf16)
        padkill = np.zeros((1, 4 * H), np.float32)
        padkill[0, H:2 * H] = PADKILL      # i-gate block (device order [o,i,f,g])
        w[f"ctlT1{d}"] = np.concatenate([bias1, padkill], 0).astype(bf16)
    w["idn"] = np.eye(H, dtype=np.float32).astype(bf16)
    w["w1Ta"] = (0.5 * inp["W1"][:, :H]).T.astype(bf16)
    w["w1Tb"] = (0.5 * inp["W1"][:, H:]).T.astype(bf16)
    w["b1row"] = inp["b1"][None, :].astype(bf16)
    w["w2Ta"] = inp["W2"][:, :H].T.astype(bf16)
    w["w2Tb"] = inp["W2"][:, H:].T.astype(bf16)
    w["b2row"] = inp["b2"][None, :].astype(bf16)
    return w


def _per_core_inputs(x, q):
    """x: [B, T, D] f32.  Builds xaug [KP, SPAN0*B] and ctl1 [2, SPAN0*B]."""
    t0 = WIN * q - WARM
    xaug = np.zeros((KP, SPAN0 * B), np.float32)
    ctl = np.zeros((2, SPAN0 * B), np.float32)
    for s in range(SPAN0):
        t = t0 + s
        sl = slice(s * B, (s + 1) * B)
        if 0 <= t < T:
            xaug[:D, sl] = x[:, t, :].T
            xaug[D, sl] = 1.0
            ctl[0, sl] = 1.0
        else:
            ctl[1, sl] = 1.0
    return xaug.astype(bf16), ctl.astype(bf16)


def _get_program():
    if "nc" not in _CACHE:
        _CACHE["nc"] = _build_program()
    return _CACHE["nc"]


def _run(inputs, trace=False):
    inp = {k: np.asarray(v) for k, v in inputs.items()}
    nc = _get_program()
    w = _prep_weights(inp)
    x = inp["x"].astype(np.float32)
    in_maps = []
    for q in range(NC):
        xaug, ctl = _per_core_inputs(x, q)
        m = dict(w)
        m["xaug"] = xaug
        m["ctl1"] = ctl
        in_maps.append(m)
    res = run_bass_kernel_spmd(nc, in_maps, list(range(NC)), trace=trace)
    outp = np.zeros((B, T, D), np.float32)
    for q in range(NC):
        o = res.results[q]["out"].reshape(WIN, B, D)        # [tok, b, d]
        outp[:, WIN * q:WIN * (q + 1), :] = o.transpose(1, 0, 2)
    return outp, res


def kernel(**inputs):
    out, _ = _run(inputs, trace=False)
    return out



# revision 14
# speedup vs baseline: 1.3314x; 1.3034x over previous
"""Trainium2 Bass kernel for 2-layer BiLSTM + classifier (nn_BiLSTM_45234595561814).

Strategy (8 NeuronCores, single SPMD launch, no collectives):
  - Each core q owns a 64-token window of T=512, FULL batch (B=64), split into
    NU=2 sub-windows of SW=32 tokens.  The two sub-windows' forward
    recurrences run as ONE lockstep "super-chain" (and both backwards as
    another): they are mutually independent, so lockstep costs no latency,
    but every engine instruction doubles its payload (gates ACT [128,512]),
    amortizing the ~352-cycle ACT pipe fill, and the hh matmuls share
    weights across sub-windows (4 matmuls of N=128 per super-step).
  - Sequence parallelism via truncated warmup: LSTM state decays ~0.5/step,
    so a chain zero-initialized WARM steps before its window converges to
    the exact state.  L0 chains span [sub-window-WARM, +SW+WARM) so L1
    warmups are fed locally; no cross-core exchange anywhere.
  - SPAN-SLOT storage: xaug/ctl/y0/y1/xg1 are all indexed by token position
    (span-slot), not by chain step.  One xaug serves both directions (B
    just reads blocks descending), and the L1 projection + classifier reads
    are contiguous N=512 matmuls.
  - One-tanh trick: i,f,o weight rows pre-scaled by 0.5 so sigmoid(z) =
    0.5*(1+tanh(z/2)) needs only tanh -> ONE ACT op for all 4 gates.
    State kept doubled (C=2c, hh=2h); cell update is 3 STT DVE ops + 1 for
    hh.  Whh pre-scaled by extra 0.5 to absorb hh=2h.  Cell state in bf16.
  - L0 input projection fused into the per-step PSUM accumulation (xaug and
    wihT0 zero-padded to K=128 for the FWL fast path).  L1 projection
    precomputed into SBUF (bf16, gate-major span-slot blocks) and injected
    into gate PSUM via bf16 identity-matmul; projection chunks are paced
    into L0-tail / L1 PE idle slots by readiness/deadline order.
  - Pad tokens (outside [0,512)) handled exactly: x/ones rows zero keep
    state at 0 through leading pads; an L1 control row drives the i-gate
    preact to -30000 on pad tokens so pad xg1 cannot perturb state.
  - Classifier interleaved into the L1 loop chunk-by-chunk as both
    directions' span-slots complete; final GEMM emitted transposed (tokens
    on partitions), tanh batched 4 span-blocks per ACT.

kernel(**inputs) takes the FULL inputs and returns the FULL [64,512,64] f32
output.  Self-contained: hardcodes all shapes; no sibling imports.
"""

import os

import numpy as np
import ml_dtypes

import concourse.bass as bass
import concourse.mybir as mybir
import concourse.tile as tile
from concourse import bacc
from concourse.bass_utils import run_bass_kernel_spmd

bf16 = ml_dtypes.bfloat16
F32, BF16 = mybir.dt.float32, mybir.dt.bfloat16
AluOp = mybir.AluOpType
ACT_TANH = mybir.ActivationFunctionType.Tanh
ACT_RELU = mybir.ActivationFunctionType.Relu

H = 128          # rnn size
B = 64           # batch
T = 512          # seq len
D = 64           # input size
NC = 8           # cores
WIN = T // NC    # tokens per core window = 64
NU = 2           # sub-windows per core
SW = WIN // NU   # sub-window size = 32
B2 = NU * B      # columns per super-slot = 128
WARM = int(os.environ.get("BILSTM_WARM", "8"))
STATE_BF16 = os.environ.get("BILSTM_STATE_BF16", "1") == "1"
S0 = SW + 2 * WARM   # L0 super-chain steps = 48
S1 = SW + WARM       # L1 super-chain steps = 40
PADKILL = -30000.0
KP = 128         # padded contraction dim for L0 inproj (D+1 -> 128, FWL)
NTOK = SW * B2   # classifier columns = 4096

_CACHE = {}


def _build_program():
    nc = bacc.Bacc(None, target_bir_lowering=False)

    # ---------------- I/O declarations ----------------
    ei = lambda name, shape, dt=BF16: nc.dram_tensor(name, shape, dt, kind="ExternalInput")
    xaug = ei("xaug", [KP, S0 * B2])   # span-slot blocks; rows 0..63 x.T, row 64 ones
    ctl = ei("ctl", [2, S0 * B2])      # row0 valid, row1 padkill indicator
    wihT0 = {d: ei(f"wihT0{d}", [KP, 4 * H]) for d in "fb"}
    whhT0 = {d: ei(f"whhT0{d}", [H, 4 * H]) for d in "fb"}
    whhT1 = {d: ei(f"whhT1{d}", [H, 4 * H]) for d in "fb"}
    wih1Ta = {d: ei(f"wih1Ta{d}", [H, 4 * H]) for d in "fb"}   # y0F K-tile
    wih1Tb = {d: ei(f"wih1Tb{d}", [H, 4 * H]) for d in "fb"}   # y0B K-tile
    ctlT1 = {d: ei(f"ctlT1{d}", [2, 4 * H]) for d in "fb"}     # bias row + padkill row
    idn = ei("idn", [H, H])
    w1Ta = ei("w1Ta", [H, 2 * H])   # (0.5*W1).T rows 0:128  -> [128, 256]
    w1Tb = ei("w1Tb", [H, 2 * H])   # rows 128:256
    b1row = ei("b1row", [1, 2 * H])
    w2Ta = ei("w2Ta", [H, D])       # W2.T rows 0:128 -> [128, 64]
    w2Tb = ei("w2Tb", [H, D])
    b2row = ei("b2row", [1, D])
    out = nc.dram_tensor("out", [NTOK, D], F32, kind="ExternalOutput")

    SDT = BF16 if STATE_BF16 else F32

    with tile.TileContext(nc) as tc:
        with tc.tile_pool(name="singles", bufs=1) as singles, \
             tc.tile_pool(name="state", bufs=1) as state, \
             tc.tile_pool(name="tpool", bufs=4) as tpool, \
             tc.tile_pool(name="vpool", bufs=3) as vpool, \
             tc.tile_pool(name="clssb", bufs=3) as clssb, \
             tc.tile_pool(name="psA", bufs=3, space="PSUM") as psA, \
             tc.tile_pool(name="psB", bufs=3, space="PSUM") as psB, \
             tc.tile_pool(name="psP", bufs=2, space="PSUM") as psP:

            # ---------------- load constants ----------------
            def load(src, shape, dt=BF16):
                t = singles.tile(shape, dt, name=src.name, tag=src.name)
                nc.sync.dma_start(out=t[:], in_=src[:])
                return t

            # xaug split into column chunks; F reads blocks ascending
            # (chunk 0 first), B descending (chunk 3 first)
            xaug_t = singles.tile([KP, S0 * B2], BF16, name="xaug", tag="xaug")
            XCH = S0 * B2 // 4
            for i in (0, 3):
                nc.sync.dma_start(out=xaug_t[:, i * XCH:(i + 1) * XCH],
                                  in_=xaug[:, i * XCH:(i + 1) * XCH])
            wihT0_t = {d: load(wihT0[d], [KP, 4 * H]) for d in "fb"}
            whhT0_t = {d: load(whhT0[d], [H, 4 * H]) for d in "fb"}
            for i in (1, 2):
                nc.sync.dma_start(out=xaug_t[:, i * XCH:(i + 1) * XCH],
                                  in_=xaug[:, i * XCH:(i + 1) * XCH])
            ctl_t = load(ctl, [2, S0 * B2])
            whhT1_t = {d: load(whhT1[d], [H, 4 * H]) for d in "fb"}
            wih1Ta_t = {d: load(wih1Ta[d], [H, 4 * H]) for d in "fb"}
            wih1Tb_t = {d: load(wih1Tb[d], [H, 4 * H]) for d in "fb"}
            ctlT1_t = {d: load(ctlT1[d], [2, 4 * H]) for d in "fb"}
            idn_t = load(idn, [H, H])
            w1Ta_t = load(w1Ta, [H, 2 * H])
            w1Tb_t = load(w1Tb, [H, 2 * H])
            b1row_t = load(b1row, [1, 2 * H])
            w2Ta_t = load(w2Ta, [H, D])
            w2Tb_t = load(w2Tb, [H, D])
            b2row_t = load(b2row, [1, D])

            # ---------------- persistent state (span-slot layouts) ----------
            y0 = {d: state.tile([H, S0 * B2], BF16, name=f"y0{d}", tag=f"y0{d}") for d in "fb"}
            y1 = {d: state.tile([H, S1 * B2], BF16, name=f"y1{d}", tag=f"y1{d}") for d in "fb"}
            h00 = state.tile([H, B2], BF16, name="h00", tag="h00")
            nc.vector.memset(h00[:], 0.0)
            # L1 projection, bf16, span-slot blocks of [4 gates x B2]
            xg1 = {d: state.tile([H, S1 * 4 * B2], BF16, name=f"xg1{d}", tag=f"xg1{d}") for d in "fb"}

            # span-slot of chain d at step s (L0 / L1)
            sp0 = lambda d, s: s if d == "f" else S0 - 1 - s
            sp1 = lambda d, s: s if d == "f" else S1 - 1 - s

            # ---------------- super-step primitives ----------------
            # gate PSUM tile: [H, 4*B2] f32 = exactly 1 bank, col = g*B2+u*B+b
            # T tile: [H, 5*B2]: tanh(gates) [o|i|f|g] then C (doubled cell)
            pend = {}     # (layer, d, step) -> (gate psum tile, T tile)
            pT = {}       # (layer, d, step) -> T tile

            def prep0(step, first=False):
                # allocate gate tile + fused input projection for L0 step
                for d in "fb":
                    ps = psA if d == "f" else psB
                    g_t = ps.tile([H, 4 * B2], F32, name="g0" + d, tag="g" + d)
                    blk = sp0(d, step) * B2
                    for g in range(4):
                        nc.tensor.matmul(g_t[:, g * B2:(g + 1) * B2],
                                         wihT0_t[d][:, g * H:(g + 1) * H],
                                         xaug_t[:, blk:blk + B2],
                                         start=(g == 0), stop=False,
                                         skip_group_check=True)
                    t_t = tpool.tile([H, 5 * B2], SDT, name="t0" + d, tag="t0" + d)
                    if first:
                        nc.vector.memset(t_t[:, 4 * B2:5 * B2], 0.0)
                    pend[(0, d, step)] = (g_t, t_t)
                    pT[(0, d, step)] = t_t

            def prep1(step, first=False):
                # allocate gate tile + identity-inject the projection for L1
                for d in "fb":
                    ps = psA if d == "f" else psB
                    g_t = ps.tile([H, 4 * B2], F32, name="g1" + d, tag="g" + d)
                    blk = sp1(d, step) * 4 * B2
                    nc.tensor.matmul(g_t[:], idn_t[:],
                                     xg1[d][:, blk:blk + 4 * B2],
                                     start=True, stop=False, skip_group_check=True)
                    t_t = tpool.tile([H, 5 * B2], SDT, name="t1" + d, tag="t1" + d)
                    if first:
                        nc.vector.memset(t_t[:, 4 * B2:5 * B2], 0.0)
                    pend[(1, d, step)] = (g_t, t_t)
                    pT[(1, d, step)] = t_t

            def super_step(layer, whh, yt, sp, step, span):
                ctx = tc.high_priority(offset=150)
                ctx.__enter__()
                hp = {}
                for d in "fb":
                    if step == 0:
                        hp[d] = h00[:]
                    else:
                        pb = sp(d, step - 1) * B2
                        hp[d] = yt[d][:, pb:pb + B2]
                for d in "fb":
                    g_t, _ = pend[(layer, d, step)]
                    for g in range(4):
                        nc.tensor.matmul(g_t[:, g * B2:(g + 1) * B2],
                                         whh[d][:, g * H:(g + 1) * H], hp[d],
                                         start=False, stop=True,
                                         skip_group_check=True)
                for d in "fb":
                    g_t, t_t = pend.pop((layer, d, step))
                    nc.scalar.activation(t_t[:, 0:4 * B2], g_t[:], ACT_TANH)
                scr = {}
                for d in "fb":
                    t_t = pT[(layer, d, step)]
                    scr[d] = vpool.tile([H, 2 * B2], SDT, name=f"s{layer}{d}", tag=f"s{layer}{d}")
                    # scr = [(1+ti)*tg | (1+tf)*C]
                    nc.vector.scalar_tensor_tensor(scr[d][:], t_t[:, B2:3 * B2], 1.0,
                                                   t_t[:, 3 * B2:5 * B2], AluOp.add, AluOp.mult)
                for d in "fb":
                    nxt = pT[(layer, d, step + 1)]
                    nc.vector.scalar_tensor_tensor(nxt[:, 4 * B2:5 * B2], scr[d][:, B2:2 * B2],
                                                   0.5, scr[d][:, 0:B2], AluOp.mult, AluOp.add)
                tc_t = {}
                for d in "fb":
                    nxt = pT[(layer, d, step + 1)]
                    tc_t[d] = vpool.tile([H, B2], SDT, name=f"c{layer}{d}", tag=f"c{layer}{d}")
                    nc.scalar.activation(tc_t[d][:], nxt[:, 4 * B2:5 * B2], ACT_TANH, scale=0.5)
                for d in "fb":
                    t_t = pT[(layer, d, step)]
                    ycol = sp(d, step) * B2
                    nc.vector.scalar_tensor_tensor(yt[d][:, ycol:ycol + B2], t_t[:, 0:B2],
                                                   1.0, tc_t[d][:], AluOp.add, AluOp.mult)
                ctx.__exit__(None, None, None)

            # ---------------- L1 projection chunks ----------------
            # chunk (d, c): span-slots [4c, 4c+4) of chain d; source y0 span
            # range offset: L1F slot s <- L0 span s; L1B slot j <- L0 span j+WARM
            NPCH = S1 // 4
            src_off = {"f": 0, "b": WARM}

            def proj_chunk(d, c):
                s0 = 4 * c
                ycol = (src_off[d] + s0) * B2
                base = s0 * 4 * B2
                xv = xg1[d][:, base:base + 4 * 4 * B2].rearrange("h (sl c) -> h sl c", sl=4)
                for g in range(4):
                    p = psP.tile([H, 4 * B2], F32, name="pp", tag="pp")
                    nc.tensor.matmul(p[:], wih1Ta_t[d][:, g * H:(g + 1) * H],
                                     y0["f"][:, ycol:ycol + 4 * B2], start=True, stop=False)
                    nc.tensor.matmul(p[:], wih1Tb_t[d][:, g * H:(g + 1) * H],
                                     y0["b"][:, ycol:ycol + 4 * B2], start=False, stop=False)
                    nc.tensor.matmul(p[:], ctlT1_t[d][:, g * H:(g + 1) * H],
                                     ctl_t[:, ycol:ycol + 4 * B2], start=False, stop=True)
                    nc.vector.tensor_copy(xv[:, :, g * B2:(g + 1) * B2],
                                          p[:].rearrange("h (sl ub) -> h sl ub", sl=4))

            # readiness (L0 step) of proj chunk (d, c): needs L0 spans
            # [off+4c, off+4c+4) from BOTH L0 chains:
            #   F chain: step = span; B chain: step = S0-1-span
            proj_items = []
            for d in "fb":
                for c in range(NPCH):
                    lo = src_off[d] + 4 * c
                    hi = lo + 3
                    ready = max(hi, S0 - 1 - lo)
                    # consumed by prep1 of chain d at L1 step: f: 4c, b: S1-1-(4c+3)
                    dl = 4 * c if d == "f" else S1 - 4 - 4 * c
                    proj_items.append((ready, dl, d, c))
            # L0-tail pacing: 1 chunk per 2 slots once ready; leftovers go to
            # the L1 loop in deadline order (the dl<=0 ones feed L1 step 0 and
            # are emitted in the prelude)
            proj_l0 = {}
            proj_rest = []
            slot = 2 * WARM + SW // 2 + 3          # first possible readiness
            for ready, dl, d, c in sorted(proj_items):
                if dl <= 0:
                    continue
                if slot < S0 - 1 and ready <= slot:
                    proj_l0[slot] = (d, c)
                    slot += 2
                else:
                    proj_rest.append((dl, d, c))
            proj_rest.sort()

            # ---------------- layer 0 ----------------
            prep0(0, first=True)
            prep0(1)
            for step in range(S0):
                if step + 2 < S0:
                    prep0(step + 2)
                if step == S0 - 1:
                    for d in "fb":
                        pT[(0, d, S0)] = tpool.tile([H, 5 * B2], SDT, name="t0" + d,
                                                    tag="t0" + d)
                super_step(0, whhT0_t, y0, sp0, step, S0)
                if step in proj_l0:
                    proj_chunk(*proj_l0[step])

            # ---------------- classifier chunks ----------------
            # chunk w (window span-slots [w, w+4)): y1F spans [w+WARM, w+WARM+4),
            # y1B spans [w, w+4); ready when both chains produced them.
            h1pool = clssb

            def cls_chunk(w):
                CH = 4 * B2
                fcol = (w + WARM) * B2
                bcol = w * B2
                h1 = [h1pool.tile([H, CH], BF16, name="h1a", tag="h1a"),
                      h1pool.tile([H, CH], BF16, name="h1b", tag="h1b")]
                for m in range(2):
                    p = psP.tile([H, CH], F32, name="pc", tag="pp")
                    nc.tensor.matmul(p[:], w1Ta_t[:, m * H:(m + 1) * H],
                                     y1["f"][:, fcol:fcol + CH], start=True, stop=False)
                    nc.tensor.matmul(p[:], w1Tb_t[:, m * H:(m + 1) * H],
                                     y1["b"][:, bcol:bcol + CH], start=False, stop=False)
                    nc.tensor.matmul(p[:], b1row_t[:, m * H:(m + 1) * H],
                                     ctl_t[0:1, fcol:fcol + CH],
                                     start=False, stop=True)
                    nc.scalar.activation(h1[m][:], p[:], ACT_RELU)
                # final GEMM transposed: out[row, d], row = span*B2 + u*B + b
                p = psP.tile([H, 4 * D], F32, name="po", tag="pp")
                for j in range(4):
                    cj = j * B2
                    nc.tensor.matmul(p[:, j * D:(j + 1) * D], h1[0][:, cj:cj + B2],
                                     w2Ta_t[:], start=True, stop=False)
                    nc.tensor.matmul(p[:, j * D:(j + 1) * D], h1[1][:, cj:cj + B2],
                                     w2Tb_t[:], start=False, stop=False)
                    nc.tensor.matmul(p[:, j * D:(j + 1) * D],
                                     ctl_t[0:1, fcol + cj:fcol + cj + B2],
                                     b2row_t[:], start=False, stop=True)
                o_t = clssb.tile([H, 4 * D], F32, name="ot", tag="ot")
                nc.scalar.activation(o_t[:], p[:], ACT_TANH)
                for j in range(4):
                    nc.sync.dma_start(out=out[(w + j) * B2:(w + j + 1) * B2, :],
                                      in_=o_t[:, j * D:(j + 1) * D])

            cls_ready = {}
            for w in range(0, SW, 4):
                r = max(w + WARM + 3, S1 - 1 - w)
                cls_ready.setdefault(r, []).append(w)

            # ---------------- layer 1 ----------------
            # prelude: the chunks each chain consumes first (dl <= 0)
            proj_chunk("f", 0)
            proj_chunk("b", NPCH - 1)
            prep1(0, first=True)
            prep1(1)
            nrest = 0
            for step in range(S1):
                if step % 2 == 0 and nrest < len(proj_rest):
                    proj_chunk(*proj_rest[nrest][1:])
                    nrest += 1
                if step + 2 < S1:
                    prep1(step + 2)
                if step == S1 - 1:
                    for d in "fb":
                        pT[(1, d, S1)] = tpool.tile([H, 5 * B2], SDT, name="t1" + d,
                                                    tag="t1" + d)
                super_step(1, whhT1_t, y1, sp1, step, S1)
                for w in cls_ready.get(step, ()):
                    cls_chunk(w)

    nc.compile()
    return nc


# ======================= host side =======================

def _prep_weights(inp):
    """Returns dict of np arrays shared by all cores (bf16).

    Gate row-blocks reordered from reference [i,f,g,o] to device [o,i,f,g];
    i,f,o rows scaled 0.5 (one-tanh trick)."""
    H_ = H
    sr = np.full((4 * H_, 1), 0.5, np.float32)
    sr[2 * H_:3 * H_] = 1.0

    def reorder(a):           # rows [i,f,g,o] -> [o,i,f,g]
        return np.concatenate([a[3 * H_:], a[:H_], a[H_:2 * H_], a[2 * H_:3 * H_]], 0)

    w = {}
    for d, tag in (("f", "0"), ("b", "1")):
        Wih, Whh = inp[f"Wih0{tag}"], inp[f"Whh0{tag}"]
        bias = inp[f"bih0{tag}"] + inp[f"bhh0{tag}"]
        wihT = reorder(np.concatenate([Wih * sr, (bias[:, None] * sr)], 1)).T  # [65, 4H]
        w[f"wihT0{d}"] = np.concatenate(
            [wihT, np.zeros((KP - D - 1, 4 * H_), np.float32)], 0).astype(bf16)
        w[f"whhT0{d}"] = reorder(Whh * sr * 0.5).T.astype(bf16)
        Wih1, Whh1 = inp[f"Wih1{tag}"], inp[f"Whh1{tag}"]
        bias1 = reorder((inp[f"bih1{tag}"] + inp[f"bhh1{tag}"])[:, None] * sr).T
        w[f"whhT1{d}"] = reorder(Whh1 * sr * 0.5).T.astype(bf16)
        w[f"wih1Ta{d}"] = reorder(Wih1[:, :H] * sr * 0.5).T.astype(bf16)
        w[f"wih1Tb{d}"] = reorder(Wih1[:, H:] * sr * 0.5).T.astype(bf16)
        padkill = np.zeros((1, 4 * H), np.float32)
        padkill[0, H:2 * H] = PADKILL      # i-gate block (device order [o,i,f,g])
        w[f"ctlT1{d}"] = np.concatenate([bias1, padkill], 0).astype(bf16)
    w["idn"] = np.eye(H, dtype=np.float32).astype(bf16)
    w["w1Ta"] = (0.5 * inp["W1"][:, :H]).T.astype(bf16)
    w["w1Tb"] = (0.5 * inp["W1"][:, H:]).T.astype(bf16)
    w["b1row"] = inp["b1"][None, :].astype(bf16)
    w["w2Ta"] = inp["W2"][:, :H].T.astype(bf16)
    w["w2Tb"] = inp["W2"][:, H:].T.astype(bf16)
    w["b2row"] = inp["b2"][None, :].astype(bf16)
    return w


def _per_core_inputs(x, q):
    """x: [B, T, D] f32.  Builds xaug [KP, S0*B2] and ctl [2, S0*B2] in
    span-slot layout: col = s*B2 + u*B + b, token = 64q + SW*u + s - WARM."""
    xaug = np.zeros((KP, S0 * B2), np.float32)
    ctl = np.zeros((2, S0 * B2), np.float32)
    for s in range(S0):
        for u in range(NU):
            t = WIN * q + SW * u + s - WARM
            sl = slice(s * B2 + u * B, s * B2 + (u + 1) * B)
            if 0 <= t < T:
                xaug[:D, sl] = x[:, t, :].T
                xaug[D, sl] = 1.0
                ctl[0, sl] = 1.0
            else:
                ctl[1, sl] = 1.0
    return xaug.astype(bf16), ctl.astype(bf16)


def _get_program():
    if "nc" not in _CACHE:
        _CACHE["nc"] = _build_program()
    return _CACHE["nc"]


def _run(inputs, trace=False):
    inp = {k: np.asarray(v) for k, v in inputs.items()}
    nc = _get_program()
    w = _prep_weights(inp)
    x = inp["x"].astype(np.float32)
    in_maps = []
    for q in range(NC):
        xaug, ctl = _per_core_inputs(x, q)
        m = dict(w)
        m["xaug"] = xaug
        m["ctl"] = ctl
        in_maps.append(m)
    res = run_bass_kernel_spmd(nc, in_maps, list(range(NC)), trace=trace)
    outp = np.zeros((B, T, D), np.float32)
    for q in range(NC):
        o = res.results[q]["out"].reshape(SW, NU, B, D)   # [span, u, b, d]
        for u in range(NU):
            outp[:, WIN * q + SW * u:WIN * q + SW * (u + 1), :] = \
                o[:, u].transpose(1, 0, 2)
    return outp, res


def kernel(**inputs):
    out, _ = _run(inputs, trace=False)
    return out
